# revision 1
# baseline (speedup 1.0000x reference)
"""FP8 semi-sparse (2:4) activation linear — Trainium2 Bass/Tile kernel.

Reference semantics:
  Wq, W_scale = rowwise fp8(e4m3fn) quant of weight      [N, K]
  Xq, X_scale = rowwise fp8(e4m3fn) quant of x           [M, K]
  Xsp         = 2:4 sparsify of Xq (keep 2 largest |.| per group of 4,
                ties -> earlier index)
  out         = (Xsp @ Wq^T) * X_scale * W_scale^T  -> bf16

Implementation notes:
  * Data-parallel over M: each of the 8 cores gets 1024 rows of x and the
    full weight; no cross-core communication.
  * TRN fp8e4 (max 240) differs from OCP e4m3fn (max 448) only in the top
    binade, so we quantize at HALF scale: u = RNE_fp8(v/2) with |v/2|<=224,
    which matches RNE to the e4m3fn grid for all normal values.  The x4 is
    folded into the output scales.
  * Quantized (halved) values live in fp8 end-to-end.  To get K onto the
    partition dim for the matmul we XBAR-transpose the fp8 data viewed as
    uint16 (XBAR is 2-byte only): the transposed tile has one K-PAIR per
    partition with the two parity bytes interleaved along the free dim; a
    deinterleave pass splits each packed tile into two parity planes.  Plane
    o of pair-tile t holds k = 2*(128*t + p) + o on partition p — a valid
    k-(pair-)tile as long as BOTH operands use the same packing (they do).
    The two planes feed one fp8 DoubleRow matmul (contraction 256).
  * 2:4 selection runs in the packed-u16 domain (fp8 magnitude bytes compare
    as unsigned ints).  Stable tie-break via 6 pairwise comparisons
    (bij = |u_i| >= |u_j|, i<j):
      keep0 = b01+b02+b03 >= 2 ; keep1 = b12+b13-b01 >= 1
      keep2 = b23-b02-b12 >= 0 ; keep3 = b03+b13+b23 <= 1
    The keep flags become a u16 byte-mask applied with one bitwise AND.
  * Software pipeline: X-path first; weight slices quantize a few N-slices
    ahead of the matmul loop so PE never starves.
"""

import numpy as np

import concourse.bass as bass
import concourse.mybir as mybir
import concourse.tile as tile
from concourse import bacc
from concourse.bass_utils import run_bass_kernel_spmd

P = 128
M_FULL, K_FULL, N_FULL = 8192, 4096, 4096
NCORES = 8
N_SLICE = 512

F32 = mybir.dt.float32
BF16 = mybir.dt.bfloat16
FP8 = mybir.dt.float8e4
U16 = mybir.dt.uint16

AX = mybir.AxisListType.X
OP = mybir.AluOpType
AF = mybir.ActivationFunctionType

W_PREFETCH = 3  # quantize weight slices this many N-slices ahead of the MMs

# out = acc' * amax_w * (amax_x * 4/448^2); acc' is the matmul of halved values
SX_CONST = float(np.float32(4.0 / (448.0 * 448.0)))


def _quant_rowtile(nc, ldpool, spool, u8pool, src_dram, row0, k):
    """Load [128, k] f32 rows; return (fp8 tile of halved quantized values,
    clamped row absmax [128,1] f32 tile)."""
    t = ldpool.tile([P, k], F32, tag="in")
    nc.sync.dma_start(t, src_dram[row0 : row0 + P, :])
    amax = spool.tile([P, 1], F32, tag="amax")
    nc.vector.tensor_reduce(amax, t, axis=AX, op=OP.max, apply_absolute_value=True)
    amax_c = spool.tile([P, 1], F32, tag="amaxc")
    nc.vector.tensor_scalar_max(amax_c, amax, 1e-12)
    rec = spool.tile([P, 1], F32, tag="rec")
    nc.vector.reciprocal(rec, amax_c)
    g = spool.tile([P, 1], F32, tag="g")
    nc.vector.tensor_scalar_mul(g, rec, 224.0)
    u8 = u8pool.tile([P, k], FP8, tag="u8")
    nc.scalar.activation(u8, t, AF.Copy, scale=g)
    return u8, amax_c


def build_nc(m_core=M_FULL // NCORES, k=K_FULL, n=N_FULL) -> bass.Bass:
    assert m_core % P == 0 and k % (2 * P) == 0 and n % N_SLICE == 0
    m_tiles = m_core // P
    kp_tiles = k // (2 * P)  # packed k-pair tiles
    n_slices = n // N_SLICE
    groups = k // 4

    nc = bacc.Bacc()
    x = nc.declare_dram_parameter("x", [m_core, k], F32, isOutput=False)
    w = nc.declare_dram_parameter("weight", [n, k], F32, isOutput=False)
    out = nc.declare_dram_parameter("out", [m_core, n], BF16, isOutput=True)

    with tile.TileContext(nc) as tc:
        with (
            tc.tile_pool(name="dram", bufs=1, space="DRAM") as dpool,
            tc.tile_pool(name="wqdram", bufs=1, space="DRAM") as wqdpool,
            tc.tile_pool(name="ld", bufs=3) as ldpool,
            tc.tile_pool(name="small", bufs=6) as spool,
            tc.tile_pool(name="u8", bufs=3) as u8pool,
            tc.tile_pool(name="persist", bufs=1) as perpool,
        ):
            xsp_dram = dpool.tile([m_core, k], FP8)        # sparsified halved Xq
            wamax_dram = dpool.tile([n], F32)              # weight row absmax
            sx4 = perpool.tile([P, m_tiles], F32)          # x epilogue scales
            wq_slices = [
                wqdpool.tile([N_SLICE, k], FP8, name=f"wq{i}", tag=f"wq{i}")
                for i in range(n_slices)
            ]

            def quant_w_slice(ns):
                for j in range(N_SLICE // P):
                    wt_idx = (N_SLICE // P) * ns + j
                    u8, amax_c = _quant_rowtile(
                        nc, ldpool, spool, u8pool, w, P * wt_idx, k
                    )
                    nc.sync.dma_start(
                        wamax_dram[P * wt_idx : P * (wt_idx + 1)], amax_c
                    )
                    nc.sync.dma_start(wq_slices[ns][P * j : P * (j + 1), :], u8)

            # ---------------- X path: quantize + 2:4 sparsify ----------------
            with (
                tc.tile_pool(name="cmp", bufs=2) as cpool,
                tc.tile_pool(name="xsp", bufs=2) as xsppool,
            ):
                for mt in range(m_tiles):
                    u8, amax_c = _quant_rowtile(nc, ldpool, spool, u8pool, x, P * mt, k)
                    nc.vector.tensor_scalar_mul(sx4[:, mt : mt + 1], amax_c, SX_CONST)

                    # magnitude bytes of (e_{2j}, e_{2j+1}) packed per u16
                    mag = cpool.tile([P, k // 2], U16, tag="mag")
                    nc.vector.tensor_scalar(
                        mag, u8.bitcast(U16), 0x7F7F, None, op0=OP.bitwise_and
                    )
                    mlo = cpool.tile([P, k // 2], U16, tag="mlo")
                    nc.vector.tensor_scalar(mlo, mag, 0x00FF, None, op0=OP.bitwise_and)
                    mhi = cpool.tile([P, k // 2], U16, tag="mhi")
                    nc.vector.tensor_scalar(
                        mhi, mag, 8, None, op0=OP.logical_shift_right
                    )
                    # group g: e0=mlo[2g] e1=mhi[2g] e2=mlo[2g+1] e3=mhi[2g+1]
                    lo = mlo.rearrange("p (g t) -> p g t", t=2)
                    hi = mhi.rearrange("p (g t) -> p g t", t=2)
                    e = {0: lo[:, :, 0], 1: hi[:, :, 0], 2: lo[:, :, 1], 3: hi[:, :, 1]}

                    b6 = cpool.tile([P, 6, groups], U16, tag="b6")
                    pairs = [(0, 1), (0, 2), (0, 3), (1, 2), (1, 3), (2, 3)]
                    bidx = {}
                    for pi, (i, j) in enumerate(pairs):
                        nc.vector.tensor_tensor(b6[:, pi, :], e[i], e[j], op=OP.is_ge)
                        bidx[(i, j)] = pi

                    def b(i, j):
                        return b6[:, bidx[(i, j)], :]

                    kk = cpool.tile([P, 4, groups], BF16, tag="kk")
                    s = cpool.tile([P, 2, groups], BF16, tag="s")
                    # keep0: b01+b02+b03 >= 2
                    nc.vector.tensor_tensor(s[:, 0, :], b(0, 1), b(0, 2), op=OP.add)
                    nc.vector.tensor_tensor(s[:, 0, :], s[:, 0, :], b(0, 3), op=OP.add)
                    nc.vector.tensor_scalar(kk[:, 0, :], s[:, 0, :], 2.0, None, op0=OP.is_ge)
                    # keep1: b12+b13-b01 >= 1
                    nc.vector.tensor_tensor(s[:, 1, :], b(1, 2), b(1, 3), op=OP.add)
                    nc.vector.tensor_tensor(s[:, 1, :], s[:, 1, :], b(0, 1), op=OP.subtract)
                    nc.vector.tensor_scalar(kk[:, 1, :], s[:, 1, :], 1.0, None, op0=OP.is_ge)
                    # keep2: b23-b02-b12 >= 0
                    nc.vector.tensor_tensor(s[:, 0, :], b(2, 3), b(0, 2), op=OP.subtract)
                    nc.vector.tensor_tensor(s[:, 0, :], s[:, 0, :], b(1, 2), op=OP.subtract)
                    nc.vector.tensor_scalar(kk[:, 2, :], s[:, 0, :], 0.0, None, op0=OP.is_ge)
                    # keep3: b03+b13+b23 <= 1
                    nc.vector.tensor_tensor(s[:, 1, :], b(0, 3), b(1, 3), op=OP.add)
                    nc.vector.tensor_tensor(s[:, 1, :], s[:, 1, :], b(2, 3), op=OP.add)
                    nc.vector.tensor_scalar(kk[:, 3, :], s[:, 1, :], 1.0, None, op0=OP.is_le)

                    # byte-mask from keep flags: u16 elem 2g <- k0*0xFF + k1*0xFF00,
                    # elem 2g+1 <- k2*0xFF + k3*0xFF00; then one AND applies 2:4.
                    mtmp = cpool.tile([P, 2, groups], F32, tag="mtmp")
                    nc.vector.tensor_scalar_mul(mtmp[:, 0, :], kk[:, 0, :], 255.0)
                    nc.vector.tensor_scalar_mul(mtmp[:, 1, :], kk[:, 2, :], 255.0)
                    mask = cpool.tile([P, k // 2], U16, tag="mask")
                    mv = mask.rearrange("p (g t) -> p g t", t=2)
                    nc.vector.scalar_tensor_tensor(
                        mv[:, :, 0], kk[:, 1, :], 65280.0, mtmp[:, 0, :],
                        op0=OP.mult, op1=OP.add,
                    )
                    nc.vector.scalar_tensor_tensor(
                        mv[:, :, 1], kk[:, 3, :], 65280.0, mtmp[:, 1, :],
                        op0=OP.mult, op1=OP.add,
                    )
                    xsp = xsppool.tile([P, k // 2], U16, tag="xsp")
                    nc.vector.tensor_tensor(
                        xsp, u8.bitcast(U16), mask, op=OP.bitwise_and
                    )
                    nc.sync.dma_start(
                        xsp_dram.bitcast(U16)[P * mt : P * (mt + 1), :], xsp
                    )

            # ---- transpose Xsp (packed u16 pairs) and deinterleave to fp8 ----
            with tc.tile_pool(name="xspT2", bufs=1) as xt2pool:
                # [P, kp_tiles, 2, m_core]: plane o of tile t holds
                # k = 2*(128*t + p) + o for output column m
                xspT2 = xt2pool.tile([P, kp_tiles, 2, m_core], FP8)
                xsp_u16 = xsp_dram.bitcast(U16)  # [m_core, k//2]
                with tc.tile_pool(name="xtmp", bufs=4) as xtmppool:
                    for t in range(kp_tiles):
                        xt_u16 = xtmppool.tile([P, m_core], U16, tag="xt")
                        nc.sync.dma_start_transpose(
                            xt_u16, xsp_u16[:, P * t : P * (t + 1)]
                        )
                        pk = xt_u16.bitcast(FP8).rearrange("p (m o) -> p m o", o=2)
                        nc.vector.tensor_copy(xspT2[:, t, 0, :], pk[:, :, 0])
                        nc.scalar.activation(xspT2[:, t, 1, :], pk[:, :, 1], AF.Copy)

                # ---------------- W path + matmul, per N slice ----------------
                with (
                    tc.tile_pool(name="wkn", bufs=8) as wknpool,
                    tc.tile_pool(name="wk2", bufs=8) as wk2pool,
                    tc.tile_pool(name="swb", bufs=2) as swbpool,
                    tc.tile_pool(name="ep", bufs=3) as eppool,
                    tc.tile_pool(name="ob", bufs=3) as obpool,
                    tc.tile_pool(name="ps", bufs=1, space="PSUM") as pspool,
                ):
                    for ns in range(min(W_PREFETCH, n_slices)):
                        quant_w_slice(ns)

                    for ns in range(n_slices):
                        if ns + W_PREFETCH < n_slices:
                            quant_w_slice(ns + W_PREFETCH)

                        swb = swbpool.tile([P, N_SLICE], F32, tag="swb")
                        nc.sync.dma_start(
                            swb,
                            wamax_dram[N_SLICE * ns : N_SLICE * (ns + 1)]
                            .unsqueeze(0)
                            .to_broadcast([P, N_SLICE]),
                        )

                        pss = [
                            pspool.tile([P, N_SLICE], F32, tag=f"ps{m}", name=f"ps{m}")
                            for m in range(m_tiles)
                        ]
                        wq_u16 = wq_slices[ns].bitcast(U16)  # [N_SLICE, k//2]
                        for t in range(kp_tiles):
                            wt_u16 = wknpool.tile([P, N_SLICE], U16, tag="wkn")
                            nc.sync.dma_start_transpose(
                                wt_u16, wq_u16[:, P * t : P * (t + 1)]
                            )
                            wk2 = wk2pool.tile([P, 2, N_SLICE], FP8, tag="wk2")
                            pk = wt_u16.bitcast(FP8).rearrange(
                                "p (n o) -> p n o", o=2
                            )
                            nc.vector.tensor_copy(wk2[:, 0, :], pk[:, :, 0])
                            nc.scalar.activation(wk2[:, 1, :], pk[:, :, 1], AF.Copy)

                            for m in range(m_tiles):
                                nc.tensor.matmul(
                                    pss[m],
                                    lhsT=xspT2[:, t, :, P * m : P * (m + 1)],
                                    rhs=wk2,
                                    perf_mode=mybir.MatmulPerfMode.DoubleRow,
                                    start=(t == 0),
                                    stop=(t == kp_tiles - 1),
                                )

                        for m in range(m_tiles):
                            ep = eppool.tile([P, N_SLICE], F32, tag="ep")
                            nc.vector.tensor_tensor(ep, pss[m], swb, op=OP.mult)
                            ob = obpool.tile([P, N_SLICE], BF16, tag="ob")
                            nc.scalar.activation(
                                ob, ep, AF.Copy, scale=sx4[:, m : m + 1]
                            )
                            nc.sync.dma_start(
                                out[
                                    P * m : P * (m + 1),
                                    N_SLICE * ns : N_SLICE * (ns + 1),
                                ],
                                ob,
                            )

    return nc


_NC = None


def kernel(x: np.ndarray, weight: np.ndarray) -> np.ndarray:
    global _NC
    if _NC is None:
        _NC = build_nc()
        _NC.finalize()  # Bacc lowering (wait legalization etc.); pjrt path expects it
    x = np.ascontiguousarray(x, dtype=np.float32)
    weight = np.ascontiguousarray(weight, dtype=np.float32)
    m_core = M_FULL // NCORES
    in_maps = [
        {"x": x[c * m_core : (c + 1) * m_core], "weight": weight}
        for c in range(NCORES)
    ]
    res = run_bass_kernel_spmd(_NC, in_maps, list(range(NCORES)))
    return np.concatenate([res.results[c]["out"] for c in range(NCORES)], axis=0)

